# revision 6
# baseline (speedup 1.0000x reference)
"""Trainium2 Bass kernel for CLinear (int8 group-quantized linear layer).

Computes out = x @ dequant(qdata, scale).T + bias where qdata is int8 with
per-(out_feature, group-of-256-in_features) symmetric scales.

Distribution: data-parallel over the 8192 activation rows (8 cores x 1024
rows); the int8 weight + scales + bias are replicated. Each core dequantizes
the weight on-device (int8 -> bf16 multiply by broadcast 1/scale), casts its
activation shard to bf16 on-device, and runs a PE-resident K=4096 matmul with
fp32 PSUM accumulation and a fused bias add on eviction.

Host-side work is layout only: transposes/reshapes so the contraction dim
lands on SBUF partitions, plus sharding/concatenation of inputs and outputs.
"""

import sys

for _p in ("/opt/trn_rl_repo",):
    if _p not in sys.path:
        sys.path.append(_p)

import numpy as np

import concourse.bacc as bacc
import concourse.mybir as mybir
import concourse.tile as tile
from concourse import bass_utils
from concourse.bass import ts

N_CORES = 8
B, S, IN_F, OUT_F = 4, 2048, 4096, 4096
M = B * S                    # 8192 total activation rows
GS = 256                     # quantization group size (in_features axis)


def _build(in_f, out_f, m_c):
    """Build the per-core Bass program.

    Per-core tensors:
      xt   f32  [in_f, m_c]   activation shard, transposed (K on rows)
      qt   int8 [in_f, out_f] weight, transposed (K on rows)
      st   f32  [g, out_f]    scales, transposed
      bias f32  [out_f]
      out  f32  [m_c, out_f]
    """
    g = in_f // GS           # number of scale groups
    n_kt = in_f // 128       # K tiles (contraction)
    oc = 512                 # output-feature chunk = matmul free dim
    n_oc = out_f // oc
    n_st = m_c // 128        # row tiles per core

    nc = bacc.Bacc("TRN2", target_bir_lowering=False, debug=False)
    xt = nc.dram_tensor("xt", [in_f, m_c], mybir.dt.float32, kind="ExternalInput")
    qt = nc.dram_tensor("qt", [in_f, out_f], mybir.dt.int8, kind="ExternalInput")
    st = nc.dram_tensor("st", [g, out_f], mybir.dt.float32, kind="ExternalInput")
    bias = nc.dram_tensor("bias", [out_f], mybir.dt.float32, kind="ExternalInput")
    out = nc.dram_tensor("out", [m_c, out_f], mybir.dt.float32, kind="ExternalOutput")

    with tile.TileContext(nc) as tc:
        with tc.tile_pool(name="const", bufs=1) as const, \
             tc.tile_pool(name="xpool", bufs=1) as xpool, \
             tc.tile_pool(name="xstage", bufs=4) as xstage, \
             tc.tile_pool(name="wpool", bufs=8) as wpool, \
             tc.tile_pool(name="qpool", bufs=8) as qpool, \
             tc.tile_pool(name="dqpool", bufs=2) as dqpool, \
             tc.tile_pool(name="biaspool", bufs=2) as biaspool, \
             tc.tile_pool(name="opool", bufs=6) as opool, \
             tc.tile_pool(name="psum", bufs=1, space="PSUM") as psum, \
             tc.tile_pool(name="dram", bufs=1, space="DRAM") as dram:

            # 1/scale in bf16, staged to DRAM so it can be partition-broadcast.
            # The [g, out_f] scales are processed through a flat [128, x] view
            # to keep the SBUF footprint at x bytes/partition.
            assert (g * out_f) % 128 == 0 and 128 % g == 0
            x_cols = (g * out_f) // 128
            st_flat = st.ap().rearrange("g (b x) -> (g b) x", b=128 // g)
            st_sb = const.tile([128, x_cols], mybir.dt.float32)
            nc.sync.dma_start(st_sb[:], st_flat)
            nc.vector.reciprocal(st_sb[:], st_sb[:])
            dqbf = const.tile([128, x_cols], mybir.dt.bfloat16)
            nc.scalar.copy(dqbf[:], st_sb[:])
            dq_dram = dram.tile([g, out_f], mybir.dt.bfloat16)
            nc.sync.dma_start(
                dq_dram[:].rearrange("g (b x) -> (g b) x", b=128 // g), dqbf[:]
            )

            # activation shard cache: bf16, SBUF-resident, filled during o==0
            xbf = xpool.tile([128, n_kt, m_c], mybir.dt.bfloat16)

            # k-outer loop: all n_st row-tiles accumulate simultaneously in
            # PSUM, so matmuls start as soon as the first x/w k-tiles land
            # instead of waiting for the whole activation shard.
            for o in range(n_oc):
                osl = ts(o, oc)
                # 1/scale rows for this o-chunk, broadcast to 128 partitions
                # (one DMA per group so dequant of group 0 starts immediately)
                dqb = dqpool.tile([128, g, oc], mybir.dt.bfloat16)
                for gi in range(g):
                    nc.sync.dma_start(
                        dqb[:, gi, :],
                        dq_dram[gi, osl].partition_broadcast(128),
                    )
                # bias chunk broadcast for the eviction adds
                bias_b = biaspool.tile([128, oc], mybir.dt.float32)
                nc.sync.dma_start(
                    bias_b[:], bias.ap()[osl].partition_broadcast(128)
                )
                pss = [
                    psum.tile([128, oc], mybir.dt.float32, name=f"ps{s}")
                    for s in range(n_st)
                ]
                for k in range(n_kt):
                    if o == 0:
                        xs = xstage.tile([128, m_c], mybir.dt.float32)
                        nc.sync.dma_start(xs[:], xt[ts(k, 128), :])
                        nc.scalar.copy(xbf[:, k, :], xs[:])
                    qtl = qpool.tile([128, oc], mybir.dt.int8)
                    nc.sync.dma_start(qtl[:], qt[ts(k, 128), osl])
                    wt = wpool.tile([128, oc], mybir.dt.bfloat16)
                    nc.vector.tensor_tensor(
                        wt[:], qtl[:], dqb[:, (k * 128) // GS, :],
                        mybir.AluOpType.mult,
                    )
                    for s in range(n_st):
                        nc.tensor.matmul(
                            pss[s][:], xbf[:, k, ts(s, 128)], wt[:],
                            start=(k == 0), stop=(k == n_kt - 1),
                        )
                for s in range(n_st):
                    ot = opool.tile([128, oc], mybir.dt.float32)
                    nc.vector.tensor_tensor(
                        ot[:], pss[s][:], bias_b[:], mybir.AluOpType.add,
                    )
                    nc.sync.dma_start(out[ts(s, 128), osl], ot[:])

    nc.compile()
    return nc


_cache = {}


def _get_nc(in_f, out_f, m_c):
    key = (in_f, out_f, m_c)
    if key not in _cache:
        _cache[key] = _build(in_f, out_f, m_c)
    return _cache[key]


def kernel(x, qdata, scale, bias, _run_kwargs=None, _shape=None):
    """x [B,S,IN_F] f32, qdata [OUT_F, G, GS] int8, scale [OUT_F, G, 1] f32,
    bias [OUT_F] f32  ->  [B,S,OUT_F] f32."""
    if _shape is None:
        b, s, in_f, out_f = B, S, IN_F, OUT_F
    else:
        b, s, in_f, out_f = _shape
    m = b * s
    m_c = m // N_CORES
    g = in_f // GS

    x = np.asarray(x, dtype=np.float32)
    qdata = np.asarray(qdata)
    scale = np.asarray(scale, dtype=np.float32)
    bias = np.asarray(bias, dtype=np.float32)

    # host-side layout prep (permutation only): contraction dim onto rows
    xt = np.ascontiguousarray(x.reshape(m, in_f).T)          # [in_f, m]
    qt = np.ascontiguousarray(
        qdata.reshape(out_f, in_f).T)                        # [in_f, out_f] int8
    st = np.ascontiguousarray(scale.reshape(out_f, g).T)     # [g, out_f]

    nc = _get_nc(in_f, out_f, m_c)

    in_maps = []
    for c in range(N_CORES):
        in_maps.append({
            "xt": np.ascontiguousarray(xt[:, c * m_c:(c + 1) * m_c]),
            "qt": qt,
            "st": st,
            "bias": bias,
        })

    res = bass_utils.run_bass_kernel_spmd(
        nc, in_maps, core_ids=list(range(N_CORES)), **(_run_kwargs or {})
    )
    out = np.concatenate([res.results[c]["out"] for c in range(N_CORES)], axis=0)
    if _run_kwargs:
        kernel.last_result = res
    return out.reshape(b, s, out_f)


# revision 10
# speedup vs baseline: 1.0598x; 1.0598x over previous
"""Trainium2 Bass kernel for CLinear (int8 group-quantized linear layer).

Computes out = x @ dequant(qdata, scale).T + bias where qdata is int8 with
per-(out_feature, group-of-256-in_features) symmetric scales.

Distribution: data-parallel over the 8192 activation rows (8 cores x 1024
rows); the int8 weight + scales + bias are replicated. Each core dequantizes
the weight on-device (int8 -> bf16 multiply by broadcast 1/scale), casts its
activation shard to bf16 on-device, and runs a PE-resident K=4096 matmul with
fp32 PSUM accumulation and a fused bias add on eviction.

Host-side work is layout only: transposes/reshapes so the contraction dim
lands on SBUF partitions, plus sharding/concatenation of inputs and outputs.
"""

import sys

for _p in ("/opt/trn_rl_repo",):
    if _p not in sys.path:
        sys.path.append(_p)

import numpy as np

import concourse.bacc as bacc
import concourse.mybir as mybir
import concourse.tile as tile
from concourse import bass_utils
from concourse.bass import ts

N_CORES = 8
B, S, IN_F, OUT_F = 4, 2048, 4096, 4096
M = B * S                    # 8192 total activation rows
GS = 256                     # quantization group size (in_features axis)


def _build(in_f, out_f, m_c):
    """Build the per-core Bass program.

    Per-core tensors:
      xt   f32  [in_f, m_c]   activation shard, transposed (K on rows)
      qt   int8 [in_f, out_f] weight, transposed (K on rows)
      st   f32  [g, out_f]    scales, transposed
      bias f32  [out_f]
      out  f32  [m_c, out_f]
    """
    g = in_f // GS           # number of scale groups
    n_kt = in_f // 128       # K tiles (contraction)
    oc = 512                 # output-feature chunk = matmul free dim
    n_oc = out_f // oc
    n_st = m_c // 128        # row tiles per core

    nc = bacc.Bacc("TRN2", target_bir_lowering=False, debug=False)
    xt = nc.dram_tensor("xt", [in_f, m_c], mybir.dt.float32, kind="ExternalInput")
    qt = nc.dram_tensor("qt", [in_f, out_f], mybir.dt.int8, kind="ExternalInput")
    st = nc.dram_tensor("st", [g, out_f], mybir.dt.float32, kind="ExternalInput")
    bias = nc.dram_tensor("bias", [out_f], mybir.dt.float32, kind="ExternalInput")
    out = nc.dram_tensor("out", [m_c, out_f], mybir.dt.float32, kind="ExternalOutput")

    with tile.TileContext(nc) as tc:
        with tc.tile_pool(name="const", bufs=1) as const, \
             tc.tile_pool(name="xpool", bufs=1) as xpool, \
             tc.tile_pool(name="xstage", bufs=4) as xstage, \
             tc.tile_pool(name="wpool", bufs=8) as wpool, \
             tc.tile_pool(name="qpool", bufs=12) as qpool, \
             tc.tile_pool(name="dqpool", bufs=3) as dqpool, \
             tc.tile_pool(name="biaspool", bufs=2) as biaspool, \
             tc.tile_pool(name="opool", bufs=6) as opool, \
             tc.tile_pool(name="psum", bufs=1, space="PSUM") as psum, \
             tc.tile_pool(name="dram", bufs=1, space="DRAM") as dram:

            # 1/scale in bf16, staged to DRAM so it can be partition-broadcast.
            # The [g, out_f] scales are processed through a flat [128, x] view
            # to keep the SBUF footprint at x bytes/partition.
            assert (g * out_f) % 128 == 0 and 128 % g == 0
            x_cols = (g * out_f) // 128
            st_flat = st.ap().rearrange("g (b x) -> (g b) x", b=128 // g)
            st_sb = const.tile([128, x_cols], mybir.dt.float32)
            nc.sync.dma_start(st_sb[:], st_flat)
            nc.vector.reciprocal(st_sb[:], st_sb[:])
            dqbf = const.tile([128, x_cols], mybir.dt.bfloat16)
            nc.scalar.copy(dqbf[:], st_sb[:])
            dq_dram = dram.tile([g, out_f], mybir.dt.bfloat16)
            nc.sync.dma_start(
                dq_dram[:].rearrange("g (b x) -> (g b) x", b=128 // g), dqbf[:]
            )

            # activation shard cache: bf16, SBUF-resident, filled during o==0
            xbf = xpool.tile([128, n_kt, m_c], mybir.dt.bfloat16)

            # Evictions run on DVE (only non-PE engine that can read PSUM);
            # output DMAs go through gpsimd's queue so their semaphore waits
            # never stall the input-DMA stream on the sync queue.
            def evict(pss, bias_b, osl):
                for s in range(n_st):
                    ot = opool.tile([128, oc], mybir.dt.float32, name="ot")
                    nc.vector.tensor_tensor(
                        ot[:], pss[s][:], bias_b[:], mybir.AluOpType.add,
                    )
                    nc.gpsimd.dma_start(out[ts(s, 128), osl], ot[:])

            # k-outer loop: all n_st row-tiles accumulate simultaneously in
            # PSUM, so matmuls start as soon as the first x/w k-tiles land
            # instead of waiting for the whole activation shard.
            prev = None
            for o in range(n_oc):
                osl = ts(o, oc)
                # 1/scale rows for this o-chunk, broadcast to 128 partitions
                # (one DMA per group so dequant of group 0 starts immediately)
                dqb = dqpool.tile([128, g, oc], mybir.dt.bfloat16)
                for gi in range(g):
                    nc.sync.dma_start(
                        dqb[:, gi, :],
                        dq_dram[gi, osl].partition_broadcast(128),
                    )
                # bias chunk broadcast for the eviction adds
                bias_b = biaspool.tile([128, oc], mybir.dt.float32)
                nc.sync.dma_start(
                    bias_b[:], bias.ap()[osl].partition_broadcast(128)
                )
                pss = [
                    psum.tile([128, oc], mybir.dt.float32, name=f"ps{s}")
                    for s in range(n_st)
                ]
                for k in range(n_kt):
                    if o == 0:
                        xs = xstage.tile([128, m_c], mybir.dt.float32)
                        nc.sync.dma_start(xs[:], xt[ts(k, 128), :])
                        nc.scalar.copy(xbf[:, k, :], xs[:])
                    qtl = qpool.tile([128, oc], mybir.dt.int8)
                    nc.sync.dma_start(qtl[:], qt[ts(k, 128), osl])
                    wt = wpool.tile([128, oc], mybir.dt.bfloat16)
                    nc.vector.tensor_tensor(
                        wt[:], qtl[:], dqb[:, (k * 128) // GS, :],
                        mybir.AluOpType.mult,
                    )
                    if k == 2 and prev is not None:
                        # software-pipelined: previous chunk's evictions are
                        # emitted here so the DVE drains them between this
                        # chunk's early dequants, freeing PSUM banks in time
                        evict(*prev)
                    for s in range(n_st):
                        nc.tensor.matmul(
                            pss[s][:], xbf[:, k, ts(s, 128)], wt[:],
                            start=(k == 0), stop=(k == n_kt - 1),
                        )
                prev = (pss, bias_b, osl)
            evict(*prev)

    nc.compile()
    return nc


_cache = {}


def _get_nc(in_f, out_f, m_c):
    key = (in_f, out_f, m_c)
    if key not in _cache:
        _cache[key] = _build(in_f, out_f, m_c)
    return _cache[key]


def kernel(x, qdata, scale, bias, _run_kwargs=None, _shape=None):
    """x [B,S,IN_F] f32, qdata [OUT_F, G, GS] int8, scale [OUT_F, G, 1] f32,
    bias [OUT_F] f32  ->  [B,S,OUT_F] f32."""
    if _shape is None:
        b, s, in_f, out_f = B, S, IN_F, OUT_F
    else:
        b, s, in_f, out_f = _shape
    m = b * s
    m_c = m // N_CORES
    g = in_f // GS

    x = np.asarray(x, dtype=np.float32)
    qdata = np.asarray(qdata)
    scale = np.asarray(scale, dtype=np.float32)
    bias = np.asarray(bias, dtype=np.float32)

    # host-side layout prep (permutation only): contraction dim onto rows
    xt = np.ascontiguousarray(x.reshape(m, in_f).T)          # [in_f, m]
    qt = np.ascontiguousarray(
        qdata.reshape(out_f, in_f).T)                        # [in_f, out_f] int8
    st = np.ascontiguousarray(scale.reshape(out_f, g).T)     # [g, out_f]

    nc = _get_nc(in_f, out_f, m_c)

    in_maps = []
    for c in range(N_CORES):
        in_maps.append({
            "xt": np.ascontiguousarray(xt[:, c * m_c:(c + 1) * m_c]),
            "qt": qt,
            "st": st,
            "bias": bias,
        })

    res = bass_utils.run_bass_kernel_spmd(
        nc, in_maps, core_ids=list(range(N_CORES)), **(_run_kwargs or {})
    )
    out = np.concatenate([res.results[c]["out"] for c in range(N_CORES)], axis=0)
    if _run_kwargs:
        kernel.last_result = res
    return out.reshape(b, s, out_f)


# revision 13
# speedup vs baseline: 1.1584x; 1.0930x over previous
"""Trainium2 Bass kernel for CLinear (int8 group-quantized linear layer).

Computes out = x @ dequant(qdata, scale).T + bias where qdata is int8 with
per-(out_feature, group-of-256-in_features) symmetric scales.

Distribution: data-parallel over the 8192 activation rows (8 cores x 1024
rows); the int8 weight + scales + bias are replicated. Each core dequantizes
the weight on-device (int8 -> bf16 multiply by broadcast 1/scale), casts its
activation shard to bf16 on-device, and runs a PE-resident K=4096 matmul with
fp32 PSUM accumulation and a fused bias add on eviction.

Host-side work is layout only: transposes/reshapes so the contraction dim
lands on SBUF partitions, plus sharding/concatenation of inputs and outputs.
"""

import sys

for _p in ("/opt/trn_rl_repo",):
    if _p not in sys.path:
        sys.path.append(_p)

import numpy as np

import concourse.bacc as bacc
import concourse.mybir as mybir
import concourse.tile as tile
from concourse import bass_utils
from concourse.bass import ts

N_CORES = 8
B, S, IN_F, OUT_F = 4, 2048, 4096, 4096
M = B * S                    # 8192 total activation rows
GS = 256                     # quantization group size (in_features axis)


def _build(in_f, out_f, m_c):
    """Build the per-core Bass program.

    Per-core tensors:
      xt   f32  [in_f, m_c]   activation shard, transposed (K on rows)
      qt   int8 [in_f, out_f] weight, transposed (K on rows)
      st   f32  [g, out_f]    scales, transposed
      bias f32  [out_f]
      out  f32  [m_c, out_f]
    """
    g = in_f // GS           # number of scale groups
    n_kt = in_f // 128       # K tiles (contraction)
    oc = 512                 # output-feature chunk = matmul free dim
    n_oc = out_f // oc
    n_st = m_c // 128        # row tiles per core

    nc = bacc.Bacc("TRN2", target_bir_lowering=False, debug=False)
    xt = nc.dram_tensor("xt", [in_f, m_c], mybir.dt.float32, kind="ExternalInput")
    qt = nc.dram_tensor("qt", [in_f, out_f], mybir.dt.int8, kind="ExternalInput")
    st = nc.dram_tensor("st", [g, out_f], mybir.dt.float32, kind="ExternalInput")
    bias = nc.dram_tensor("bias", [out_f], mybir.dt.float32, kind="ExternalInput")
    out = nc.dram_tensor("out", [m_c, out_f], mybir.dt.float32, kind="ExternalOutput")

    with tile.TileContext(nc) as tc:
        with tc.tile_pool(name="const", bufs=1) as const, \
             tc.tile_pool(name="xpool", bufs=1) as xpool, \
             tc.tile_pool(name="xstage", bufs=4) as xstage, \
             tc.tile_pool(name="wpool", bufs=8) as wpool, \
             tc.tile_pool(name="qpool", bufs=12) as qpool, \
             tc.tile_pool(name="dqpool", bufs=3) as dqpool, \
             tc.tile_pool(name="biaspool", bufs=2) as biaspool, \
             tc.tile_pool(name="opool", bufs=8) as opool, \
             tc.tile_pool(name="psum", bufs=1, space="PSUM") as psum, \
             tc.tile_pool(name="dram", bufs=1, space="DRAM") as dram:

            # 1/scale in bf16, staged to DRAM so it can be partition-broadcast.
            # The [g, out_f] scales are processed through a flat [128, x] view
            # to keep the SBUF footprint at x bytes/partition.
            assert (g * out_f) % 128 == 0 and 128 % g == 0
            x_cols = (g * out_f) // 128
            st_flat = st.ap().rearrange("g (b x) -> (g b) x", b=128 // g)
            st_sb = const.tile([128, x_cols], mybir.dt.float32)
            nc.sync.dma_start(st_sb[:], st_flat)
            nc.vector.reciprocal(st_sb[:], st_sb[:])
            dqbf = const.tile([128, x_cols], mybir.dt.bfloat16)
            nc.scalar.copy(dqbf[:], st_sb[:])
            dq_dram = dram.tile([g, out_f], mybir.dt.bfloat16)
            nc.gpsimd.dma_start(
                dq_dram[:].rearrange("g (b x) -> (g b) x", b=128 // g), dqbf[:]
            )

            # activation shard cache: bf16, SBUF-resident, filled during o==0
            xbf = xpool.tile([128, n_kt, m_c], mybir.dt.bfloat16)

            # Evictions run on DVE (only non-PE engine that can read PSUM);
            # output DMAs go through gpsimd's queue so their semaphore waits
            # never stall the input-DMA stream on the sync queue.
            def evict(pss, bias_b, osl):
                for s in range(n_st):
                    ot = opool.tile([128, oc], mybir.dt.float32, name="ot")
                    nc.vector.tensor_tensor(
                        ot[:], pss[s][:], bias_b[:], mybir.AluOpType.add,
                    )
                    nc.gpsimd.dma_start(out[ts(s, 128), osl], ot[:])

            def emit_prep(o):
                """dqb broadcasts (gpsimd queue, so the dq-roundtrip wait and
                the 2MB of broadcast traffic never block sync's x/q stream)
                + bias chunk, emitted in the k-direction chunk o will use."""
                osl = ts(o, oc)
                dqb = dqpool.tile([128, g, oc], mybir.dt.bfloat16, name="dqb")
                gseq = range(g) if o % 2 == 0 else range(g - 1, -1, -1)
                for gi in gseq:
                    nc.gpsimd.dma_start(
                        dqb[:, gi, :],
                        dq_dram[gi, osl].partition_broadcast(128),
                    )
                bias_b = biaspool.tile([128, oc], mybir.dt.float32, name="bias_b")
                nc.sync.dma_start(
                    bias_b[:], bias.ap()[osl].partition_broadcast(128)
                )
                return dqb, bias_b

            # k-outer loop with snaked k-direction: chunk o+1 starts on the
            # k-tile chunk o finished with, so its matmuls are never gated on
            # the far end of the activation load. All n_st row-tiles
            # accumulate simultaneously in PSUM so matmuls start as soon as
            # the first x/w k-tiles land.
            prep = emit_prep(0)
            next_prep = None
            prev = None
            prep_idx = min(8, n_kt - 1)
            for o in range(n_oc):
                osl = ts(o, oc)
                dqb, bias_b = prep
                pss = [
                    psum.tile([128, oc], mybir.dt.float32, name=f"ps{s}")
                    for s in range(n_st)
                ]
                kseq = range(n_kt) if o % 2 == 0 else range(n_kt - 1, -1, -1)
                for idx, k in enumerate(kseq):
                    if o == 0:
                        xs = xstage.tile([128, m_c], mybir.dt.float32)
                        nc.sync.dma_start(xs[:], xt[ts(k, 128), :])
                        nc.scalar.copy(xbf[:, k, :], xs[:])
                    qtl = qpool.tile([128, oc], mybir.dt.int8)
                    nc.sync.dma_start(qtl[:], qt[ts(k, 128), osl])
                    wt = wpool.tile([128, oc], mybir.dt.bfloat16)
                    nc.vector.tensor_tensor(
                        wt[:], qtl[:], dqb[:, (k * 128) // GS, :],
                        mybir.AluOpType.mult,
                    )
                    if idx == 2 and prev is not None:
                        # software-pipelined: previous chunk's evictions are
                        # emitted here so the DVE drains them between this
                        # chunk's early dequants, freeing PSUM banks in time
                        evict(*prev)
                    if idx == prep_idx and o + 1 < n_oc:
                        next_prep = emit_prep(o + 1)
                    for s in range(n_st):
                        nc.tensor.matmul(
                            pss[s][:], xbf[:, k, ts(s, 128)], wt[:],
                            start=(idx == 0), stop=(idx == n_kt - 1),
                        )
                prev = (pss, bias_b, osl)
                prep = next_prep
            evict(*prev)

    nc.compile()
    return nc


_cache = {}


def _get_nc(in_f, out_f, m_c):
    key = (in_f, out_f, m_c)
    if key not in _cache:
        _cache[key] = _build(in_f, out_f, m_c)
    return _cache[key]


def kernel(x, qdata, scale, bias, _run_kwargs=None, _shape=None):
    """x [B,S,IN_F] f32, qdata [OUT_F, G, GS] int8, scale [OUT_F, G, 1] f32,
    bias [OUT_F] f32  ->  [B,S,OUT_F] f32."""
    if _shape is None:
        b, s, in_f, out_f = B, S, IN_F, OUT_F
    else:
        b, s, in_f, out_f = _shape
    m = b * s
    m_c = m // N_CORES
    g = in_f // GS

    x = np.asarray(x, dtype=np.float32)
    qdata = np.asarray(qdata)
    scale = np.asarray(scale, dtype=np.float32)
    bias = np.asarray(bias, dtype=np.float32)

    # host-side layout prep (permutation only): contraction dim onto rows
    xt = np.ascontiguousarray(x.reshape(m, in_f).T)          # [in_f, m]
    qt = np.ascontiguousarray(
        qdata.reshape(out_f, in_f).T)                        # [in_f, out_f] int8
    st = np.ascontiguousarray(scale.reshape(out_f, g).T)     # [g, out_f]

    nc = _get_nc(in_f, out_f, m_c)

    in_maps = []
    for c in range(N_CORES):
        in_maps.append({
            "xt": np.ascontiguousarray(xt[:, c * m_c:(c + 1) * m_c]),
            "qt": qt,
            "st": st,
            "bias": bias,
        })

    res = bass_utils.run_bass_kernel_spmd(
        nc, in_maps, core_ids=list(range(N_CORES)), **(_run_kwargs or {})
    )
    out = np.concatenate([res.results[c]["out"] for c in range(N_CORES)], axis=0)
    if _run_kwargs:
        kernel.last_result = res
    return out.reshape(b, s, out_f)
